# revision 14
# baseline (speedup 1.0000x reference)
"""Trainium2 Bass kernel for CrossAttention (B=4, N=2048, C=768, H=12).

Sharding: 8 cores = 4 head-groups (3 heads each) x 2 batch-groups (2 batches
each). Every core computes, for its (heads, batches):
    Q/K/V projections -> S^T = K @ Q^T + bias^T (PSUM-injected via identity
    matmul, or DVE pre-exp add for shed chunks) -> exp -> PV (ones-augmented
    V gives softmax sums free) -> normalize -> partial output projection.
Host pre-transposes inputs to [.., C|m, N] layouts and converts to bf16;
host sums the 4 head-group partial outputs and adds the projection bias.

Perf structure: attention runs in two half-column passes so the score PSUM
ring is 3 x [128,1024] (deep PE runway) with only 2 banks of PV accumulators.
Projection / output-projection matmuls for the next/previous batch are woven
as filler between attention chunks so the tensor engine never idles (it only
reaches full clock while continuously busy). Bias DMA is prefetched.
"""

import sys

for _p in ("/opt/trn_rl_repo",):
    if _p not in sys.path:
        sys.path.insert(0, _p)

import numpy as np
import ml_dtypes

B, N, C, H, D = 4, 2048, 768, 12, 64
SCALE = D ** -0.5
HG, BG = 4, 2            # head-groups x batch-groups = 8 cores
HL = H // HG             # 3 heads per core
BL = B // BG             # 2 batches per core
MT = N // 128            # 16 m tiles
CT = C // 128            # 6 c tiles
NH = 2                   # half-column passes (1024 cols each)
BF16 = ml_dtypes.bfloat16
GD = HL * D              # 192

# per-chunk bias path, cycled over (pass, mt) chunks: 'p' = PE identity
# inject, 'v' = DVE pre-exp add (sheds PE columns onto the vector engine)
SHED_PATTERN = "p"
BIAS_PREF = 4            # bias DMA prefetch depth (chunks)
PV_LAG = 2               # m-tiles of slack between exp and PV consumption

_prog_cache = {}


def _build_program():
    import concourse.bass as bass
    import concourse.tile as tile
    from concourse import bacc, mybir
    from concourse.tile_rust import add_dep_helper

    f32 = mybir.dt.float32
    bf16 = mybir.dt.bfloat16

    nc = bacc.Bacc("TRN2", target_bir_lowering=False, debug=False)

    xT = nc.dram_tensor("xT", [BL, C, N], bf16, kind="ExternalInput")
    kT = nc.dram_tensor("kT", [BL, C, N], bf16, kind="ExternalInput")
    vT = nc.dram_tensor("vT", [BL, C, N], bf16, kind="ExternalInput")
    bT = nc.dram_tensor("bT", [HL, N, N], bf16, kind="ExternalInput")  # [h, m, n]
    wq = nc.dram_tensor("wq", [C, GD], bf16, kind="ExternalInput")
    wk = nc.dram_tensor("wk", [C, GD], bf16, kind="ExternalInput")
    wv = nc.dram_tensor("wv", [C, GD], bf16, kind="ExternalInput")
    wp = nc.dram_tensor("wp", [GD, C], bf16, kind="ExternalInput")
    ones = nc.dram_tensor("ones", [1, D], bf16, kind="ExternalInput")
    ident = nc.dram_tensor("ident", [128, 128], bf16, kind="ExternalInput")
    yT = nc.dram_tensor("yT", [BL, C, N], bf16, kind="ExternalOutput")

    with tile.TileContext(nc) as tc:
        with (
            tc.tile_pool(name="wpool", bufs=1) as wpool,
            tc.tile_pool(name="stream", bufs=12) as stream,
            tc.tile_pool(name="persist", bufs=1) as persist,
            tc.tile_pool(name="biasp", bufs=6) as biasp,
            tc.tile_pool(name="ppool", bufs=10) as ppool,
            tc.tile_pool(name="miscp", bufs=1) as miscp,
            tc.tile_pool(name="miscr", bufs=2) as miscr,
            tc.tile_pool(name="ypool", bufs=3) as ypool,
            tc.tile_pool(name="ps", bufs=3, space="PSUM") as ps,
            tc.tile_pool(name="po", bufs=2, space="PSUM") as po,
        ):
            # ---- constants / weights ----
            wq_sb = wpool.tile([128, CT * GD], bf16, tag="wq")
            nc.sync.dma_start(wq_sb.rearrange("p (t d) -> p t d", d=GD),
                              wq.rearrange("(t p) d -> p t d", p=128))
            wk_sb = wpool.tile([128, CT * GD], bf16, tag="wk")
            nc.sync.dma_start(wk_sb.rearrange("p (t d) -> p t d", d=GD),
                              wk.rearrange("(t p) d -> p t d", p=128))
            wv_sb = wpool.tile([128, CT * GD], bf16, tag="wv")
            nc.sync.dma_start(wv_sb.rearrange("p (t d) -> p t d", d=GD),
                              wv.rearrange("(t p) d -> p t d", p=128))
            wp0_sb = wpool.tile([128, C], bf16, tag="wp0")
            nc.sync.dma_start(wp0_sb[:], wp[0:128, :])
            wp1_sb = wpool.tile([64, C], bf16, tag="wp1")
            nc.sync.dma_start(wp1_sb[:], wp[128:192, :])
            ones_sb = wpool.tile([1, D], bf16, tag="ones")
            nc.sync.dma_start(ones_sb[:], ones[:, :])
            id_sb = wpool.tile([128, 128], bf16, tag="ident")
            nc.sync.dma_start(id_sb[:], ident[:, :])

            # head groups: heads 0,1 packed in 128 partitions; head 2 in 64
            groups = [(0, 128), (128, 64)]

            # ---- persistent per-batch tensors ----
            qT01, qT2, kT01, kT2 = {}, {}, {}, {}
            on01, on2 = {}, {}
            vall = {}
            for b in range(BL):
                qT01[b] = persist.tile([128, N], bf16, tag=f"q01_{b}", name=f"q01_{b}")
                qT2[b] = persist.tile([64, N], bf16, tag=f"q2_{b}", name=f"q2_{b}")
                kT01[b] = persist.tile([128, N], bf16, tag=f"k01_{b}", name=f"k01_{b}")
                kT2[b] = persist.tile([64, N], bf16, tag=f"k2_{b}", name=f"k2_{b}")
                on01[b] = persist.tile([128, N], bf16, tag=f"on01_{b}", name=f"on01_{b}")
                on2[b] = persist.tile([64, N], bf16, tag=f"on2_{b}", name=f"on2_{b}")
                # V for all 3 heads: [m, (t, h, D+1)] with ones at c=D
                vall[b] = persist.tile([128, MT * HL * (D + 1)], bf16,
                                       tag=f"v_{b}", name=f"v_{b}")
                v4 = vall[b].rearrange("p (t h c) -> p t h c", h=HL, c=D + 1)
                nc.gpsimd.memset(v4[:, :, :, D], 1.0)

            def v_slice(b, h, mt):
                off = (mt * HL + h) * (D + 1)
                return vall[b][:, off:off + (D + 1)]

            # ================= projection thunks =================
            def proj_thunks(b):
                """Closures, each emitting one PSUM-tile's worth of projection
                work for batch b. None entries are spacer slots that give the
                stream DMAs a head start."""
                tiles = {}

                def dma_in(name, src):
                    def go():
                        for ct in range(CT):
                            t = stream.tile([128, N], bf16, tag="stream",
                                            name="stream_t")
                            nc.gpsimd.dma_start(
                                t[:], src[b, ct * 128:(ct + 1) * 128, :])
                            tiles[(name, ct)] = t
                    return go

                def qk_tile(name, w_sb, dst01, dst2, goff, gsz, nb):
                    def go():
                        pq = ps.tile([128, 1024], f32, tag="s", name="ps_s")
                        for hf in range(2):
                            for ct in range(CT):
                                nc.tensor.matmul(
                                    pq[0:gsz, hf * 512:(hf + 1) * 512],
                                    w_sb[:, ct * GD + goff: ct * GD + goff + gsz],
                                    tiles[(name, ct)][:, nb * 1024 + hf * 512:
                                                      nb * 1024 + (hf + 1) * 512],
                                    start=(ct == 0), stop=(ct == CT - 1))
                        dst = dst01 if gsz == 128 else dst2
                        nc.vector.tensor_copy(
                            dst[:, nb * 1024:(nb + 1) * 1024], pq[0:gsz, :])
                    return go

                def v_tile(mt):
                    def go():
                        pv = ps.tile([128, 1024], f32, tag="s", name="ps_v")
                        for ct in range(CT):
                            nc.tensor.matmul(
                                pv[:, 0:GD],
                                tiles[("v", ct)][:, mt * 128:(mt + 1) * 128],
                                wv_sb[:, ct * GD:(ct + 1) * GD],
                                start=(ct == 0), stop=(ct == CT - 1))
                        for h in range(HL):
                            nc.vector.tensor_copy(
                                v_slice(b, h, mt)[:, 0:D],
                                pv[:, h * D:(h + 1) * D])
                    return go

                th = [dma_in("q", xT), dma_in("k", kT), dma_in("v", vT)]
                th += [None] * 9
                for name, w_sb, d01, d2 in (("q", wq_sb, qT01[b], qT2[b]),
                                            ("k", wk_sb, kT01[b], kT2[b])):
                    for goff, gsz in groups:
                        for nb in range(2):
                            th.append(qk_tile(name, w_sb, d01, d2,
                                              goff, gsz, nb))
                for mt in range(MT):
                    th.append(v_tile(mt))
                return th

            # ================= output-projection thunks =================
            def outproj_thunks(b):
                th = []
                y_sb = {}

                def mm_tile(ct, nb):
                    def go():
                        if nb == 0:
                            y_sb[ct] = ypool.tile([128, N], bf16, tag="y",
                                                  name="y_t")
                        py = ps.tile([128, 1024], f32, tag="s", name="ps_y")
                        for hf in range(2):
                            sl = slice(nb * 1024 + hf * 512,
                                       nb * 1024 + (hf + 1) * 512)
                            nc.tensor.matmul(
                                py[:, hf * 512:(hf + 1) * 512],
                                wp0_sb[:, ct * 128:(ct + 1) * 128],
                                on01[b][:, sl], start=True, stop=False)
                            nc.tensor.matmul(
                                py[:, hf * 512:(hf + 1) * 512],
                                wp1_sb[:, ct * 128:(ct + 1) * 128],
                                on2[b][:, sl], start=False, stop=True)
                        nc.vector.tensor_copy(
                            y_sb[ct][:, nb * 1024:(nb + 1) * 1024], py[:])
                        if nb == 1:
                            nc.gpsimd.dma_start(
                                yT[b, ct * 128:(ct + 1) * 128, :], y_sb[ct][:])
                    return go

                for ct in range(CT):
                    for nb in range(2):
                        th.append(mm_tile(ct, nb))
                return th

            # ================= attention =================
            def attention(b, h, filler, fill_per_mt):
                """Attention for (b, h): two passes over 1024-col halves.
                Drains up to fill_per_mt filler thunks per (pass, mt) iter."""
                if h < 2:
                    k_src = kT01[b][h * D:(h + 1) * D, :]
                    q_src = qT01[b][h * D:(h + 1) * D, :]
                else:
                    k_src = kT2[b][:, :]
                    q_src = qT2[b][:, :]

                bts = {}
                chunk_no = [0]

                def bias_dma(half, mt):
                    bt = biasp.tile([128, 1024], bf16, tag="bias",
                                    name="bias_t")
                    nc.sync.dma_start(
                        bt[:], bT[h, mt * 128:(mt + 1) * 128,
                                  half * 1024:(half + 1) * 1024])
                    bts[(half, mt)] = bt

                def prefetch(idx):
                    if idx < NH * MT:
                        bias_dma(idx // MT, idx % MT)

                for i in range(BIAS_PREF):
                    prefetch(i)

                for half in range(NH):
                    pos = [po.tile([D + 1, 512], f32, tag="o", name="po_o")
                           for _ in range(2)]
                    pts = {}
                    for mt in range(MT + PV_LAG):
                        if mt < MT:
                            prefetch(half * MT + mt + BIAS_PREF)
                            bt = bts.pop((half, mt))
                            sp = ps.tile([128, 1024], f32, tag="s",
                                         name="ps_sc")
                            mode = SHED_PATTERN[chunk_no[0] % len(SHED_PATTERN)]
                            chunk_no[0] += 1
                            injs = []
                            for hf in range(2):
                                off = half * 1024 + hf * 512
                                qk_i = nc.tensor.matmul(
                                    sp[:, hf * 512:(hf + 1) * 512],
                                    k_src[:, mt * 128:(mt + 1) * 128],
                                    q_src[:, off:off + 512],
                                    start=True, stop=(mode == "v"))
                                if mode == "p":
                                    inj_i = nc.tensor.matmul(
                                        sp[:, hf * 512:(hf + 1) * 512],
                                        id_sb[:],
                                        bt[:, hf * 512:(hf + 1) * 512],
                                        start=False, stop=True)
                                    add_dep_helper(inj_i.ins, qk_i.ins,
                                                   reason="bias after scores")
                                    injs.append(inj_i)
                            pt = ppool.tile([128, 1024], bf16, tag="p",
                                            name="p_t")
                            if mode == "v":
                                # DVE adds bias in-place in PSUM (no SBUF
                                # staging traffic); exp then reads PSUM
                                nc.vector.tensor_add(sp[:], sp[:], bt[:])
                            exp_i = nc.scalar.activation(
                                pt[:], sp[:],
                                mybir.ActivationFunctionType.Exp)
                            for inj_i in injs:
                                add_dep_helper(exp_i.ins, inj_i.ins,
                                               reason="exp after bias")
                            pts[mt] = pt
                        if mt >= PV_LAG:
                            pm = mt - PV_LAG
                            vsl = v_slice(b, h, pm)
                            pt = pts.pop(pm)
                            for hf in range(2):
                                nc.tensor.matmul(
                                    pos[hf][:], vsl,
                                    pt[:, hf * 512:(hf + 1) * 512],
                                    start=(pm == 0), stop=(pm == MT - 1))
                        for _ in range(fill_per_mt):
                            t = next(filler, None)
                            if t is not None:
                                t()

                    # ---- normalization for this half (staged off PSUM
                    # fast; math trails on ACT/PE/GpSimd so the DVE queue
                    # stays clear and the next half's PV can start) ----
                    ost = miscr.tile([D, 1024], bf16, tag="ost", name="ost")
                    sum_sb = miscp.tile([1, 1024], f32, tag="sum_sb",
                                        name="sum_sb")
                    rec_f = miscp.tile([1, 1024], f32, tag="rec_f",
                                       name="rec_f")
                    rec_b = miscp.tile([1, 1024], bf16, tag="rec_b",
                                       name="rec_b")
                    for hf in range(2):
                        nc.vector.tensor_copy(
                            ost[:, hf * 512:(hf + 1) * 512], pos[hf][0:D, :])
                        nc.vector.tensor_copy(
                            sum_sb[:, hf * 512:(hf + 1) * 512],
                            pos[hf][D:D + 1, :])
                    nc.vector.reciprocal_approx_fast(rec_f[:], sum_sb[:])
                    nc.scalar.copy(rec_b[:], rec_f[:])
                    if h < 2:
                        dst0 = on01[b][h * D:(h + 1) * D, :]
                    else:
                        dst0 = on2[b][:, :]
                    r_ps = ps.tile([128, 1024], f32, tag="s", name="ps_r")
                    for hf in range(2):
                        nc.tensor.matmul(
                            r_ps[0:D, hf * 512:(hf + 1) * 512], ones_sb[:],
                            rec_b[:, hf * 512:(hf + 1) * 512],
                            start=True, stop=True)
                    r_sb = miscr.tile([D, 1024], bf16, tag="r_sb",
                                      name="r_sb")
                    nc.vector.tensor_copy(r_sb[:], r_ps[0:D, :])
                    for hf in range(2):
                        sl = slice(half * 1024 + hf * 512,
                                   half * 1024 + (hf + 1) * 512)
                        nc.vector.tensor_mul(
                            dst0[:, sl], ost[:, hf * 512:(hf + 1) * 512],
                            r_sb[:, hf * 512:(hf + 1) * 512])

            # ================= schedule =================
            empty = iter(())
            for t in proj_thunks(0):
                if t is not None:
                    t()
            pt1 = iter(proj_thunks(1))
            op0 = None
            for b in range(BL):
                for h in range(HL):
                    if b == 0:
                        attention(b, h, pt1, 1)
                    elif b == 1 and h == 0:
                        for t in pt1:
                            if t is not None:
                                t()
                        op0 = iter(outproj_thunks(0))
                        attention(b, h, op0, 1)
                    elif b == 1 and h == 1:
                        attention(b, h, op0, 1)
                    else:
                        for t in op0:
                            t()
                        attention(b, h, empty, 0)
            for t in outproj_thunks(1):
                t()

    nc.compile()
    return nc


def get_program():
    if "nc" not in _prog_cache:
        _prog_cache["nc"] = _build_program()
    return _prog_cache["nc"]


def make_in_maps(x, k_in, v_in, rel_pos_bias, Wq, Wk, Wv, Wp):
    xT = x.transpose(0, 2, 1).astype(BF16)
    kT = k_in.transpose(0, 2, 1).astype(BF16)
    vT = v_in.transpose(0, 2, 1).astype(BF16)
    bT = rel_pos_bias.transpose(0, 2, 1).astype(BF16)       # [H, m, n]
    WqT = (Wq * SCALE).T.astype(BF16)                       # [C, C]
    WkT = Wk.T.astype(BF16)
    WvT = Wv.T.astype(BF16)
    WpT = Wp.T.astype(BF16)                                 # [C(d_in), C]
    ones = np.ones((1, D), dtype=BF16)
    ident = np.eye(128, dtype=BF16)

    in_maps = []
    for c in range(8):
        hg, bg = c % HG, c // HG
        hs, bs = hg * HL, bg * BL
        in_maps.append({
            "xT": np.ascontiguousarray(xT[bs:bs + BL]),
            "kT": np.ascontiguousarray(kT[bs:bs + BL]),
            "vT": np.ascontiguousarray(vT[bs:bs + BL]),
            "bT": np.ascontiguousarray(bT[hs:hs + HL]),
            "wq": np.ascontiguousarray(WqT[:, hs * D:(hs + HL) * D]),
            "wk": np.ascontiguousarray(WkT[:, hs * D:(hs + HL) * D]),
            "wv": np.ascontiguousarray(WvT[:, hs * D:(hs + HL) * D]),
            "wp": np.ascontiguousarray(WpT[hs * D:(hs + HL) * D, :]),
            "ones": ones,
            "ident": ident,
        })
    return in_maps


def assemble_output(results, bp):
    y = np.zeros((B, C, N), dtype=np.float32)
    for c in range(8):
        hg, bg = c % HG, c // HG
        bs = bg * BL
        y[bs:bs + BL] += results[c]["yT"].astype(np.float32)
    out = y.transpose(0, 2, 1) + bp.astype(np.float32)
    return np.ascontiguousarray(out.astype(np.float32))


def kernel(**inputs):
    from concourse.bass_utils import run_bass_kernel_spmd

    x = np.asarray(inputs["x"], dtype=np.float32)
    k_in = np.asarray(inputs["k_in"], dtype=np.float32)
    v_in = np.asarray(inputs["v_in"], dtype=np.float32)
    rel_pos_bias = np.asarray(inputs["rel_pos_bias"], dtype=np.float32)
    Wq = np.asarray(inputs["Wq"], dtype=np.float32)
    Wk = np.asarray(inputs["Wk"], dtype=np.float32)
    Wv = np.asarray(inputs["Wv"], dtype=np.float32)
    Wp = np.asarray(inputs["Wp"], dtype=np.float32)
    bp = np.asarray(inputs["bp"], dtype=np.float32)

    nc = get_program()
    in_maps = make_in_maps(x, k_in, v_in, rel_pos_bias, Wq, Wk, Wv, Wp)
    res = run_bass_kernel_spmd(nc, in_maps, list(range(8)))
    return assemble_output(res.results, bp)
